# revision 14
# baseline (speedup 1.0000x reference)
"""Trainium2 Bass kernel for nn_BoothGroupQuant.

Booth/NAF group quantization: q = rne(x*128); NAF-decompose each q into
signed power-of-two digits; per group of 16 consecutive elements keep only
the 8 largest-exponent digits (ties: lower exponent first, then element
order); reconstruct and scale by 1/128.

Core identity: with t = 3q, u = t ^ q, the NAF nonzero-digit mask of q is u
(digit at exponent e <-> bit e+1), positive digits at u & t, negative at
u & q -- valid directly on two's-complement negatives.

Three-pass structure so the per-group ("tiny") arithmetic runs once on
[P, 256] tiles instead of per-chunk on [P, 64..96] tiles (per-op overhead
amortization):
  P1 (per chunk): q, u, SWAR band counts, grouped reduce -> R4
  tiny-A (once): band sums, h-tests, bstar/theta
  P2 (per chunk): amt broadcast, spread, segmented scan -> PM4, TPv -> TP4
  tiny-B (once): packed per-exp thresholds -> TH4
  P3 (per chunk): guard-bit compare, keep mask, reconstruct, output
Design range |q| <= 2730.
"""
import os
import sys

import numpy as np

for _p in ("/opt/trn_rl_repo", "/root/.axon_site/_ro/trn_rl_repo"):
    if os.path.isdir(_p) and _p not in sys.path:
        sys.path.insert(0, _p)

import concourse.bacc as bacc
import concourse.mybir as mybir
from concourse import bass_utils
from concourse.tile import TileContext

N_CORES = 8
FULL_SHAPE = (4, 1024, 32, 32)
N_TOTAL = 4 * 1024 * 32 * 32          # 4194304
N_CORE = N_TOTAL // N_CORES           # 524288
P = 128                               # SBUF partitions
F_TOTAL = N_CORE // P                 # 4096 free elems per partition
G_TOTAL = F_TOTAL // 16               # 256 groups per partition
CHUNKS = (512, 1536, 1536, 512)
F_CHUNK = max(CHUNKS)
SF = 0.0078125

i16 = mybir.dt.int16
i8 = mybir.dt.int8
f32 = mybir.dt.float32
Alu = mybir.AluOpType
Act = mybir.ActivationFunctionType
AX = mybir.AxisListType

_CACHE = {}


def grp(ap):
    return ap.rearrange("p (g s) -> p g s", s=16)


def _build():
    nc = bacc.Bacc("TRN2")
    x_in = nc.dram_tensor("x", [P, F_TOTAL], f32, kind="ExternalInput")
    y_out = nc.dram_tensor("y", [P, F_TOTAL], f32, kind="ExternalOutput")
    V, S = nc.vector, nc.scalar

    with TileContext(nc) as tc:
        with tc.tile_pool(name="persist", bufs=1) as pp:
            # segment mask: 0 at each group start, 1 elsewhere
            seg = pp.tile([P, F_CHUNK], i16)
            V.memset(seg, 32767)
            V.memset(grp(seg)[:, :, 0:1], 0)

            U4 = pp.tile([P, F_TOTAL], i16, name="U4")
            Q4 = pp.tile([P, F_TOTAL], i16, name="Q4")
            PM4 = pp.tile([P, F_TOTAL], i16, name="PM4")
            AM4 = pp.tile([P, F_TOTAL], i16, name="AM4")
            R4 = pp.tile([P, 2 * G_TOTAL], i16, name="R4")   # [RE all | RO all]
            TP4 = pp.tile([P, G_TOTAL], i16, name="TP4")
            HS4 = pp.tile([P, G_TOTAL], i16, name="HS4")
            TT4 = pp.tile([P, G_TOTAL], i16, name="TT4")     # theta
            TH4 = pp.tile([P, G_TOTAL], i16, name="TH4")     # packed thresholds

            with tc.tile_pool(name="work", bufs=2) as pool:
                # ---- pass 1: counts + reduce ----
                off = 0
                for fc in (256, 1280, 1536, 1024):
                    _p1(nc, pool, x_in, U4, Q4, R4, off, fc)
                    off += fc

                # ---- tiny-A: band sums -> hs, theta ----
                def tiny(nm):
                    return pool.tile([P, G_TOTAL], i16, name=nm, tag=nm)

                RO = R4[:, G_TOTAL:]
                H2 = pool.tile([P, 2 * G_TOTAL], i16, name="H2", tag="H2")
                V.tensor_scalar(H2, R4, 6, None, Alu.logical_shift_right)
                B2 = H2[:, 0:G_TOTAL]
                B3 = H2[:, G_TOTAL:]
                B1 = tiny("B1")
                V.tensor_scalar(B1, RO, 63, None, Alu.bitwise_and)
                s2 = tiny("s2")
                V.tensor_tensor(s2, B3, B2, Alu.add)
                s1 = tiny("s1")
                V.tensor_tensor(s1, s2, B1, Alu.add)
                h3 = tiny("h3")
                V.tensor_scalar(h3, B3, 8, None, Alu.is_lt)
                h2 = tiny("h2")
                V.tensor_scalar(h2, s2, 8, None, Alu.is_lt)
                h1 = tiny("h1")
                V.tensor_scalar(h1, s1, 8, None, Alu.is_lt)
                V.tensor_tensor(HS4, h3, h2, Alu.add)
                V.tensor_tensor(HS4, HS4, h1, Alu.add)
                c3 = tiny("c3")
                V.tensor_tensor(c3, B3, h3, Alu.mult)
                c2 = tiny("c2")
                V.tensor_tensor(c2, B2, h2, Alu.mult)
                c1 = tiny("c1")
                V.tensor_tensor(c1, B1, h1, Alu.mult)
                V.tensor_tensor(c3, c3, c2, Alu.add)
                V.tensor_tensor(c3, c3, c1, Alu.add)       # = Cab
                V.tensor_scalar(TT4, c3, -1, 8, Alu.mult, Alu.add)  # theta

                # ---- pass 2: spread + segmented scan ----
                off = 0
                goff = 0
                for fc in CHUNKS:
                    _p2(nc, pool, seg, U4, AM4, PM4, TP4, HS4, off, fc, goff)
                    off += fc
                    goff += fc // 16

                # ---- tiny-B: packed thresholds ----
                n2 = tiny("n2")
                V.tensor_scalar(n2, TP4, 10, 31, Alu.logical_shift_right,
                                Alu.bitwise_and)
                n1 = tiny("n1")
                V.tensor_scalar(n1, TP4, 5, 31, Alu.logical_shift_right,
                                Alu.bitwise_and)
                th1 = tiny("th1")
                V.tensor_tensor(th1, TT4, n2, Alu.subtract)
                th0 = tiny("th0")
                V.tensor_tensor(th0, th1, n1, Alu.subtract)
                th1c = tiny("th1c")
                V.tensor_scalar(th1c, th1, 0, 32, Alu.max, Alu.mult)
                th0c = tiny("th0c")
                V.tensor_scalar(th0c, th0, 0, None, Alu.max)
                t2s = tiny("t2s")
                V.tensor_scalar(t2s, TT4, 1024, None, Alu.mult)
                V.tensor_tensor(th0c, th0c, th1c, Alu.add)
                V.tensor_tensor(TH4, th0c, t2s, Alu.add)

                # ---- pass 3: compare + keep + reconstruct ----
                off = 0
                goff = 0
                for fc in (1536, 1536, 768, 256):
                    _p3(nc, pool, y_out, U4, Q4, PM4, AM4, TH4, off, fc, goff)
                    off += fc
                    goff += fc // 16

    nc.compile()
    return nc


def _p1(nc, pool, x_in, U4, Q4, R4, off, fc):
    Fc = fc
    Gc = Fc // 16
    goff = off // 16
    sl = slice(off, off + fc)
    V, S = nc.vector, nc.scalar

    def full(nm, dt=i16, nb=None):
        return pool.tile([P, Fc], dt, name=nm, tag=nm, bufs=nb)

    xt = full("xt", f32, nb=2)
    nc.sync.dma_start(out=xt, in_=x_in[:, sl])
    q = Q4[:, sl]
    S.activation(q, xt, Act.Copy, scale=128.0)
    t = full("t")
    V.tensor_scalar(t, q, 3, None, Alu.mult)
    u = U4[:, sl]
    V.tensor_tensor(u, t, q, Alu.bitwise_xor)

    A = full("A")
    V.tensor_scalar(A, u, 1, 0x249, Alu.logical_shift_right, Alu.bitwise_and)
    B = full("B")
    V.tensor_scalar(B, u, 2, 0x249, Alu.logical_shift_right, Alu.bitwise_and)
    C = full("C")
    V.tensor_scalar(C, u, 3, 0x249, Alu.logical_shift_right, Alu.bitwise_and)
    V.tensor_tensor(A, A, B, Alu.add)
    V.tensor_tensor(A, A, C, Alu.add)
    D = pool.tile([P, 2 * Fc], i16, name="D", tag="D")
    V.tensor_scalar(D[:, 0:Fc], A, 0x1C7, None, Alu.bitwise_and)
    V.tensor_scalar(D[:, Fc:], A, 3, 0x1C7, Alu.logical_shift_right,
                    Alu.bitwise_and)

    # reduce into R4 slices: RE -> [goff:goff+Gc], RO -> [G_TOTAL+goff:...]
    Rv = R4.rearrange("p (k g) -> p k g", k=2)[:, :, goff:goff + Gc]
    with nc.allow_low_precision(reason="exact small int sums"):
        V.tensor_reduce(Rv, D.rearrange("p (k g s) -> p k g s", s=16, k=2),
                        AX.X, Alu.add)


def _p2(nc, pool, seg, U4, AM4, PM4, TP4, HS4, off, fc, goff):
    Fc = fc
    Gc = Fc // 16
    sl = slice(off, off + fc)
    V, S = nc.vector, nc.scalar

    def full(nm, dt=i16, nb=None):
        return pool.tile([P, Fc], dt, name=nm, tag=nm, bufs=nb)

    def bc(tiny_ap):
        return tiny_ap[:, :, None].broadcast_to((P, Gc, 16))

    amtx = AM4[:, sl]
    S.activation(grp(amtx), bc(HS4[:, goff:goff + Gc]), Act.Copy,
                 scale=-3.0, bias=10.0)
    w = full("w")
    V.tensor_tensor(w, U4[:, sl], amtx, Alu.logical_shift_right)
    s = full("s")
    V.tensor_scalar(s, w, 7, None, Alu.bitwise_and)
    sm = full("sm")
    V.tensor_scalar(sm, s, 273, None, Alu.mult)
    V.tensor_scalar(s, sm, 0x421, None, Alu.bitwise_and)
    Pm = PM4[:, sl]
    V.tensor_tensor_scan(Pm, seg[:, 0:Fc], s, 0.0, Alu.min, Alu.add)
    # group-last (packed per-exp totals) -> contiguous TP4 slice
    V.tensor_scalar(TP4[:, goff:goff + Gc], grp(Pm)[:, :, 15], 0, None,
                    Alu.add)


def _p3(nc, pool, y_out, U4, Q4, PM4, AM4, TH4, off, fc, goff):
    Fc = fc
    Gc = Fc // 16
    sl = slice(off, off + fc)
    V, S = nc.vector, nc.scalar

    def full(nm, dt=i16, nb=None):
        return pool.tile([P, Fc], dt, name=nm, tag=nm, bufs=nb)

    def bc(tiny_ap):
        return tiny_ap[:, :, None].broadcast_to((P, Gc, 16))

    thx = full("thx", nb=2)
    S.activation(grp(thx), bc(TH4[:, goff:goff + Gc]), Act.Copy,
                 bias=float(0x4210))
    X = full("X")
    V.tensor_tensor(X, thx, PM4[:, sl], Alu.subtract)
    s1g = full("s1g")
    V.tensor_scalar(s1g, X, 12, 4, Alu.logical_shift_right, Alu.bitwise_and)
    ga = full("ga")
    V.tensor_scalar(ga, X, 4, 0x421, Alu.logical_shift_right, Alu.bitwise_and)
    gm2 = full("gm2", nb=2)
    # floor(ga*17/16) == (ga*17)>>4 exactly (frac = a/16 < 0.5, RNE floors)
    S.activation(gm2, ga, Act.Copy, scale=17.0 / 16.0)
    k1 = full("k1")
    V.tensor_tensor(k1, s1g, gm2, Alu.bitwise_or)
    V.tensor_scalar(k1, k1, -8, None, Alu.bitwise_or)
    Ku = full("Ku")
    V.tensor_tensor(Ku, k1, AM4[:, sl], Alu.logical_shift_left)
    u = U4[:, sl]
    q = Q4[:, sl]
    V.tensor_tensor(u, u, Ku, Alu.bitwise_and)                 # UK
    V.tensor_tensor(q, u, q, Alu.bitwise_and)                  # NM
    V.tensor_scalar(q, q, 2, None, Alu.mult)
    V.tensor_tensor(u, u, q, Alu.subtract)                     # val
    yt = full("yt", f32, nb=2)
    S.activation(yt, u, Act.Copy, scale=SF / 2.0)
    nc.sync.dma_start(out=y_out[:, sl], in_=yt)


def _get_nc():
    if "nc" not in _CACHE:
        _CACHE["nc"] = _build()
    return _CACHE["nc"]


def kernel(x: np.ndarray, _trace: bool = False, _trace_kwargs=None):
    assert x.shape == FULL_SHAPE and x.dtype == np.float32, (x.shape, x.dtype)
    nc = _get_nc()
    flat = np.ascontiguousarray(x).reshape(N_CORES, P, F_TOTAL)
    in_maps = [{"x": flat[i]} for i in range(N_CORES)]
    kw = {}
    if _trace:
        kw = {"trace": True, **(_trace_kwargs or {})}
    res = bass_utils.run_bass_kernel_spmd(
        nc, in_maps, core_ids=list(range(N_CORES)), **kw)
    out = np.stack([res.results[i]["y"] for i in range(N_CORES)], axis=0)
    out = out.reshape(FULL_SHAPE).astype(np.float32)
    if _trace:
        return out, res
    return out
